# revision 61
# baseline (speedup 1.0000x reference)
"""Multi-head attention (b=4, n=2048, dm=1024, h=16) on 8 TRN2 NeuronCores.

Sharding: batch (4) x head-group (2) -> 8 cores, Megatron-style.
Core c handles batch c//2 and heads [8*(c%2), 8*(c%2)+8).

v5 design (HW 380us; v3 was 440us, v1 616us).  Trace-driven findings:
  * Stage 2 is jointly limited by PE matmul streaming (~225us of
    N-cycle work), exp throughput (33.5M elements across scalar+DVE),
    and the scores->exp->scores PSUM-slot dependency ring.  v3 ran the
    ring with 2 slots and 4:1 exp split: warm clock but exp-bound
    (~300us).  v5 uses 256-query iterations so a score psum bank holds
    [128, 2 heads x 256q] and a [128,1024] 2-bank tile covers TWO
    key-chunks: exp stays a big paired instruction (engine overhead
    ~220ns amortized over 1024 cols) AND the score pool holds 3 slots
    (6 banks) + 1-bank ctx accumulators x2 = 8 banks exactly.  Scores
    only wait on exp three pairs back; ctx matmuls trail scores by two
    pairs (deeper trailing measured worse).
  * exp split 5:3 scalar:DVE (bf16 Schraudolph int16 bit trick on DVE),
    interleaved s,s,d,s,d,s,s,d so the scalar never builds a backlog.
    Every softmax row sees a uniform 3/8 trick share (rel err 1.03e-2
    vs the 2e-2 gate; all-fp8/DoubleRow variants measured 2-5e-2 in a
    host-side quantization study -- attention output here is a
    near-uniform average, std ~1/45 of v, so e4m3 noise lands ~1:1 on
    the output; fp8 is unusable everywhere in this problem).
  * Softmax denominator: ones column FIRST in v' (padded to 128 cols:
    [ones, 63 zeros, v0..63]) so the denominator lands on psum
    PARTITION 0 -- the custom-DVE reciprocal_approx_fast and the
    gpsimd partition_broadcast ucode both silently require base
    partition 0 on real HW (verified; partition-64 inputs return
    garbage).  Chain per iteration: DVE row copy + reciprocal, gpsimd
    broadcast to all 128 partitions, then two DVE multiplies (psum rows
    64:128, a legal DVE base-64 window) + SBUF->SBUF partition-shift
    DMAs into ctx_sb.  Nothing here touches the PE or the score pool,
    and the chain is short enough that the multiplies never wait (a
    late rec head-of-line-blocks the DVE exp queue -- measured 2us
    stalls with the earlier DMA-shift chain).
  * HW gotchas baked in: two CONCURRENT row-tiled matmuls (the K=64
    head pair at tile rows 0/64) must write DIFFERENT psum banks or
    the device hangs; one psum accumulation group per 2KB bank
    (zero-region granularity); each dma_start costs ~3us of queue
    time, so stage-1 inputs ship as a few fat DMAs (x quarter in one,
    weights f-chunk-major so matmul #1 only waits for x + 1/8 of wqk).
  * Stage 1 f32r QKV matmuls from bf16-bit inputs; psum pairs
    [128,1024] so each psum->bf16 cast covers two matmul outputs,
    alternating scalar/DVE.  Stage 3 bf16 with one 2-bank psum per
    token block, consecutive matmuls sharing their stationary ctx
    chunk, single [128,1024] cast, one output DMA per block.
Host sums the two partials per batch and adds the bias.
"""

import numpy as np

import concourse.bass as bass
import concourse.tile as tile
from concourse import bacc, library_config, mybir
from concourse import bass_utils

f32 = mybir.dt.float32
f32r = mybir.dt.float32r
f8 = mybir.dt.float8e4
bf16 = mybir.dt.bfloat16
i16 = mybir.dt.int16
u16 = mybir.dt.uint16
Exp = mybir.ActivationFunctionType.Exp
Copy = mybir.ActivationFunctionType.Copy
Mult = mybir.AluOpType.mult
Add = mybir.AluOpType.add

TOK = 2048          # tokens per batch
DM = 1024           # model dim
DL = 512            # local q/k/v feature dim (8 heads x 64)
D = 64              # head dim
NH = 8              # local heads
NPAIR = 4           # head pairs (partition blocks of ctx/qk)
KT = 8              # dm / 128 contraction tiles
SCALE = DM ** (-0.5)
N_CORES = 8

# bf16 Schraudolph: bf16bits(exp(s*SCALE)) ~= trunc(s * A16 + B16).
# The -6.75 zeroes the trick's mean multiplicative bias (so scalar-exp'd
# and trick'd key-chunks weight consistently inside one softmax row);
# it splits the difference between truncating (-6.5) and rounding (-7.0)
# f32->i16 conversion.
A16 = 128.0 * SCALE / float(np.log(2.0))
B16 = 127.0 * 128.0 - 6.75

# exp engine schedule, cycled per PAIRED exp tile ([128,1024] covering
# TWO key-chunks x both heads at 256-query granularity): 5/8 scalar,
# 3/8 DVE, which balances the engines (DVE also carries the softmax
# division) and keeps every softmax row's Schraudolph share a uniform
# 3/8.  (gpsimd cannot access PSUM on TRN2, so it cannot help w/ exp.)
EXP_SCHED = ("s", "s", "d", "s", "d", "s", "s", "d")


def _build(tc, xT, wqkT, wvT, wo16, out_p):
    nc = tc.nc
    # gpsimd ucode library with partition_broadcast (softmax denominator)
    nc.gpsimd.load_library(library_config.attn)

    w3p = tc.alloc_tile_pool(name="w3", bufs=1)
    ctp = tc.alloc_tile_pool(name="ctp", bufs=1)
    qkp = tc.alloc_tile_pool(name="qkp", bufs=1)
    vp_ = tc.alloc_tile_pool(name="vp", bufs=1)

    ctx_sb = ctp.tile([128, NPAIR, TOK], bf16, tag="ctx")    # 16 KB/part
    qk_sb = qkp.tile([128, 2 * NPAIR, TOK], bf16, tag="qk")  # 32 KB/part
    # v' columns: [ones, 31 zero-pad, v0..v63] = 96.  The ones column in
    # position 0 puts the softmax denominator on psum PARTITION 0 (legal
    # base for the custom-DVE reciprocal and the gpsimd broadcast, which
    # silently require it on HW); ctx rows land at 32:96, handled by the
    # division as two 32-partition windows (DVE base-32 ops are capped
    # at 32 partitions).  96 weight columns keep the ctx LDWEIGHTS off
    # the critical path (vs 128-col padding).
    v_sb = vp_.tile([128, 16, NH, 96], bf16, tag="v")        # 24 KB/part
    # output-projection weights prefetched up front so stage 3 never
    # waits on DMA
    wout_sb = w3p.tile([128, NPAIR, DM], u16, tag="wout")    # 8 KB/part
    nc.sync.dma_start(wout_sb[:], wo16[:])

    # ones column FIRST in v' (softmax denominator accumulator): the
    # denominator then lands on psum PARTITION 0, where the custom-DVE
    # reciprocal and the gpsimd partition_broadcast actually work on HW
    # (both silently require base partition 0) -- no DMA shift needed
    # on the latency-critical reciprocal chain
    nc.vector.memset(v_sb[:, :, :, 0:1], 1.0)
    nc.vector.memset(v_sb[:, :, :, 1:32], 0.0)

    # ---- stage 1: QKV projection in f32r, token-quarters of 512 ----
    # psum tiles are PAIRED [128,1024] (2 banks) so every psum->bf16 cast
    # covers two matmul outputs: the ~220ns per-instruction engine
    # overhead is paid half as often on both DVE and scalar.
    with (
        tc.tile_pool(name="w1", bufs=1) as wp,
        tc.tile_pool(name="x", bufs=2) as xp,
        tc.tile_pool(name="psq", bufs=2, space="PSUM") as psq,
    ):
        # wqkT is staged f-chunk-major [128, 8(f), KT, 128] so the first
        # q/k matmul group only waits for x plus 1/8 of the weights
        wqk_sb = wp.tile([128, 8, KT, 128], u16, tag="wqk")   # 16 KB/part
        wv_sb = wp.tile([128, KT, DL], u16, tag="wv")         # 8 KB/part
        xt0 = xp.tile([128, KT, 512], u16, tag="x")
        # few, fat DMAs: each dma_start costs ~3us of queue time here, so
        # batch everything except the f0 weight chunk (which gates MM #1)
        nc.sync.dma_start(xt0[:], xT[:, :, 0:512])
        nc.sync.dma_start(wqk_sb[:, 0], wqkT[:, 0])
        nc.sync.dma_start(wqk_sb[:, 1:8], wqkT[:, 1:8])
        nc.sync.dma_start(wv_sb[:], wvT[:])
        ncast = 0
        for tq in range(4):
            ts512 = slice(tq * 512, (tq + 1) * 512)
            if tq == 0:
                xt = xt0
            else:
                xt = xp.tile([128, KT, 512], u16, tag="x")
                nc.sync.dma_start(xt[:], xT[:, :, ts512])
            # q/k (transposed layout): out [feat 128, tok 512], f-pairs
            for fp in range(4):
                ps = psq.tile([128, 1024], f32, tag="qk", bufs=3)
                for half in range(2):
                    f = 2 * fp + half
                    hs = slice(half * 512, (half + 1) * 512)
                    for kt in range(KT):
                        nc.tensor.matmul(
                            ps[:, hs],
                            wqk_sb[:, f, kt, :].bitcast(bf16),
                            xt[:, kt, :].bitcast(bf16),
                            start=(kt == 0), stop=(kt == KT - 1))
                with nc.allow_low_precision(reason="bf16"):
                    dst = qk_sb[:, 2 * fp:2 * fp + 2, ts512]
                    src = ps[:].rearrange("p (two t) -> p two t", two=2)
                    if ncast % 2 == 0:
                        nc.vector.tensor_copy(dst, src)
                    else:
                        nc.scalar.activation(dst, src, Copy)
                ncast += 1
            # v (natural layout): out [tok 128, feat 512], tt-pairs
            for tp in range(2):
                ps = psq.tile([128, 1024], f32, tag="v", bufs=1)
                for half in range(2):
                    tt = 2 * tp + half
                    hs = slice(half * 512, (half + 1) * 512)
                    for kt in range(KT):
                        nc.tensor.matmul(
                            ps[:, hs],
                            xt[:, kt, tt * 128:(tt + 1) * 128].bitcast(bf16),
                            wv_sb[:, kt, :].bitcast(bf16),
                            start=(kt == 0), stop=(kt == KT - 1))
                jt0 = tq * 4 + 2 * tp
                with nc.allow_low_precision(reason="bf16 v"):
                    nc.vector.tensor_copy(
                        v_sb[:, jt0:jt0 + 2, :, 32:96],
                        ps[:].rearrange("p (two h d) -> p two h d",
                                        two=2, h=NH))

    # ---- stage 2: attention, per head-pair, query-eighths of 256 ----
    with (
        tc.tile_pool(name="psS", bufs=3, space="PSUM") as psS,
        tc.tile_pool(name="psC", bufs=2, space="PSUM") as psC,
        tc.tile_pool(name="ep", bufs=8) as ep,
        tc.tile_pool(name="dv", bufs=2) as dv,
    ):
        eidx = 0

        def div_tail(ps_c, rec, ihs, p):
            # softmax division for both heads of the pair.  rec (rows
            # 64:128 = broadcast reciprocal denominators) produced at
            # the end of the accumulating iteration with a short chain
            # (DVE copy + DVE recip + gpsimd broadcast, all at their
            # native partitions), so these multiplies never wait and
            # never head-of-line-block the DVE's exp queue.  ctx lives
            # on psum rows 64:128; both heads shift partitions via cheap
            # off-critical-path SBUF->SBUF DMAs.
            with nc.allow_low_precision(reason="bf16 ctx"):
                for hh in (0, 1):
                    tmp = dv.tile([96, 256], bf16, tag="tmp",
                                  name=f"tmp{hh}")
                    for r0 in (32, 64):
                        nc.vector.tensor_tensor(
                            out=tmp[r0:r0 + 32, :],
                            in0=ps_c[r0:r0 + 32, hh * 256:(hh + 1) * 256],
                            in1=rec[r0:r0 + 32, hh * 256:(hh + 1) * 256],
                            op=Mult)
                    nc.sync.dma_start(
                        ctx_sb[64 * hh:64 * hh + 64, p, ihs], tmp[32:96, :])

        pending_div = None   # previous iteration's division, deferred to
        # the middle of the next iteration (all off the PE critical path)
        # 256-query iterations: score psum per key-chunk is ONE bank
        # ([128, 2 heads x 256q]) so a [128,1024] 2-bank tile covers TWO
        # key-chunks -> exp stays a paired instruction AND the score pool
        # holds 3 slots: scores(jp) only wait on exp(jp-3), deep enough
        # that neither exp latency nor engine jitter ever stalls the PE.
        for p in range(4):
            for i8 in range(8):
                ihs = slice(i8 * 256, (i8 + 1) * 256)
                # both heads' ctx accumulators in one 1-bank psum tile
                ps_c = psC.tile([96, 512], f32, tag="C")

                def ctx_mm(jp_c, e_kc):
                    # ONE psum accumulation group for the whole bank (the
                    # zero-region granularity is a 2KB bank): start on the
                    # very first matmul, stop on the very last.  The ctx
                    # matmuls use full-height lhsT so they never run
                    # concurrently -> same-bank writes are safe here.
                    for sub in (0, 1):
                        kc = 2 * jp_c + sub
                        for hh in (0, 1):
                            nc.tensor.matmul(
                                ps_c[:, hh * 256:(hh + 1) * 256],
                                v_sb[:, kc, 2 * p + hh, :],
                                e_kc[:, hh * 512 + sub * 256:
                                     hh * 512 + (sub + 1) * 256],
                                start=(kc == 0 and hh == 0),
                                stop=(kc == 15 and hh == 1))

                e_hist = []   # E tiles whose ctx matmuls are still pending
                for jp in range(8):
                    ps_s = psS.tile([128, 1024], f32, tag="S")
                    # bank 0 = head a (sub0|sub1 columns), bank 1 = head b:
                    # the two heads' row-tiled matmuls run CONCURRENTLY in
                    # the PE, so they must write different psum banks (a
                    # same-bank concurrent write hangs the device); the
                    # same-head/same-bank sub matmuls are one group each
                    for sub in (0, 1):
                        jt = 2 * jp + sub
                        js = slice(jt * 128, (jt + 1) * 128)
                        for poff, hh in ((0, 0), (64, 1)):
                            nc.tensor.matmul(
                                ps_s[:, hh * 512 + sub * 256:
                                     hh * 512 + (sub + 1) * 256],
                                qk_sb[poff:poff + D, 4 + p, js],
                                qk_sb[poff:poff + D, p, ihs],
                                start=(sub == 0), stop=(sub == 1))
                    if jp >= 2:
                        ctx_mm(jp - 2, e_hist.pop(0))
                    if jp == 4 and pending_div is not None:
                        div_tail(*pending_div)
                        pending_div = None
                    e_t = ep.tile([128, 1024], bf16, tag="E")
                    eng = EXP_SCHED[eidx % len(EXP_SCHED)]
                    eidx += 1
                    with nc.allow_low_precision(reason="exp"):
                        if eng == "s":
                            nc.scalar.activation(e_t[:], ps_s[:], Exp,
                                                 scale=SCALE)
                        else:
                            nc.vector.tensor_scalar(
                                e_t[:].bitcast(i16), ps_s[:],
                                A16, B16, Mult, Add)
                    e_hist.append(e_t)
                # drain the last two key-chunk pairs' ctx
                ctx_mm(6, e_hist[0])
                ctx_mm(7, e_hist[1])
                # denominators sit on psum ROW 0 (ones column is first in
                # v'): copy out, reciprocal, and broadcast all operate at
                # base partition 0 natively -- short low-latency chain
                den0 = dv.tile([1, 512], f32, tag="den0")
                nc.vector.tensor_copy(den0[:], ps_c[0:1, :])
                rec0 = dv.tile([1, 512], f32, tag="rec0")
                nc.vector.reciprocal_approx_fast(rec0[:], den0[:])
                rec = dv.tile([96, 512], f32, tag="rec")
                nc.gpsimd.partition_broadcast(rec[:], rec0[:], channels=96)
                pending_div = (ps_c, rec, ihs, p)
        div_tail(*pending_div)

    vp_.release()
    qkp.release()

    # ---- stage 3: output projection (bf16) ----
    # one 2-bank psum per token-block; consecutive matmuls share their
    # stationary ctx chunk, and the psum->f32 cast is a single [128,1024]
    # instruction alternating between scalar and DVE
    with (
        tc.tile_pool(name="psO", bufs=2, space="PSUM") as psO,
        tc.tile_pool(name="ot", bufs=3) as otp,
    ):
        for tt in range(16):
            o_t = otp.tile([128, DM], f32, tag="o")
            ps = psO.tile([128, 1024], f32, tag="O")
            for pb in range(NPAIR):
                for fc in range(2):
                    fs = slice(fc * 512, (fc + 1) * 512)
                    nc.tensor.matmul(
                        ps[:, fs], ctx_sb[:, pb, tt * 128:(tt + 1) * 128],
                        wout_sb[:, pb, fs].bitcast(bf16),
                        start=(pb == 0), stop=(pb == NPAIR - 1))
            if tt % 2 == 0:
                nc.scalar.activation(o_t[:], ps[:], Copy)
            else:
                nc.vector.tensor_copy(o_t[:], ps[:])
            nc.sync.dma_start(out_p[tt * 128:(tt + 1) * 128, :], o_t[:])
    ctp.release()
    w3p.release()


_CACHE = {}


def _get_nc():
    if "nc" not in _CACHE:
        nc = bacc.Bacc("TRN2", target_bir_lowering=False, debug=False)
        xT = nc.dram_tensor("xT", [128, KT, TOK], u16, kind="ExternalInput").ap()
        wqkT = nc.dram_tensor("wqkT", [128, 8, KT, 128], u16,
                              kind="ExternalInput").ap()
        wvT = nc.dram_tensor("wvT", [128, KT, DL], u16,
                             kind="ExternalInput").ap()
        wo16 = nc.dram_tensor("wo16", [128, NPAIR, DM], u16,
                              kind="ExternalInput").ap()
        out_p = nc.dram_tensor("out_p", [TOK, DM], f32, kind="ExternalOutput").ap()
        with tile.TileContext(nc) as tc:
            _build(tc, xT, wqkT, wvT, wo16, out_p)
        nc.compile()
        _CACHE["nc"] = nc
    return _CACHE["nc"]


def _bf16_bits(x):
    b = np.ascontiguousarray(np.asarray(x, np.float32)).view(np.uint32)
    return ((b + 0x7FFF + ((b >> 16) & 1)) >> 16).astype(np.uint16)


def _fold(a):
    """[DM, cols] -> [128, KT, cols] bf16 bits, partition-major dm chunks."""
    b = _bf16_bits(a)
    return np.ascontiguousarray(
        b.reshape(KT, 128, a.shape[1]).transpose(1, 0, 2))


def make_in_maps(x, w_qkv, w_out):
    in_maps = []
    xTb = {b: _fold(np.ascontiguousarray(x[b].T)) for b in range(4)}
    for c in range(N_CORES):
        b, g = c // 2, c % 2
        gs = slice(g * DL, (g + 1) * DL)
        wq = w_qkv[0 * DM + g * DL:0 * DM + (g + 1) * DL]
        wk = w_qkv[1 * DM + g * DL:1 * DM + (g + 1) * DL]
        wv = w_qkv[2 * DM + g * DL:2 * DM + (g + 1) * DL]
        woT = np.ascontiguousarray(w_out[:, gs].T)        # [DL, DM]
        wo16 = np.ascontiguousarray(
            _bf16_bits(woT).reshape(NPAIR, 128, DM).transpose(1, 0, 2))
        wqk_fold = _fold(np.ascontiguousarray(np.concatenate([wq, wk], 0).T))
        # [128, KT, 1024] -> f-chunk-major [128, 8, KT, 128]
        wqk_fold = np.ascontiguousarray(
            wqk_fold.reshape(128, KT, 8, 128).transpose(0, 2, 1, 3))
        in_maps.append({
            "xT": xTb[b],
            "wqkT": wqk_fold,
            "wvT": _fold(np.ascontiguousarray(wv.T)),
            "wo16": wo16,
        })
    return in_maps


def kernel(x, w_qkv, w_out, b_out, _trace=False):
    x = np.asarray(x, dtype=np.float32)
    w_qkv = np.asarray(w_qkv, dtype=np.float32)
    w_out = np.asarray(w_out, dtype=np.float32)
    b_out = np.asarray(b_out, dtype=np.float32)

    nc = _get_nc()
    in_maps = make_in_maps(x, w_qkv, w_out)
    res = bass_utils.run_bass_kernel_spmd(
        nc, in_maps, core_ids=list(range(N_CORES)), trace=_trace)
    out = np.empty((4, TOK, DM), dtype=np.float32)
    for b in range(4):
        out[b] = res.results[2 * b]["out_p"] + res.results[2 * b + 1]["out_p"]
    out += b_out
    if _trace:
        kernel.last_results = res
    return out



# revision 62
# speedup vs baseline: 1.0195x; 1.0195x over previous
"""Multi-head attention (b=4, n=2048, dm=1024, h=16) on 8 TRN2 NeuronCores.

Sharding: batch (4) x head-group (2) -> 8 cores, Megatron-style.
Core c handles batch c//2 and heads [8*(c%2), 8*(c%2)+8).

v5 design (HW 380us; v3 was 440us, v1 616us).  Trace-driven findings:
  * Stage 2 is jointly limited by PE matmul streaming (~225us of
    N-cycle work), exp throughput (33.5M elements across scalar+DVE),
    and the scores->exp->scores PSUM-slot dependency ring.  v3 ran the
    ring with 2 slots and 4:1 exp split: warm clock but exp-bound
    (~300us).  v5 uses 256-query iterations so a score psum bank holds
    [128, 2 heads x 256q] and a [128,1024] 2-bank tile covers TWO
    key-chunks: exp stays a big paired instruction (engine overhead
    ~220ns amortized over 1024 cols) AND the score pool holds 3 slots
    (6 banks) + 1-bank ctx accumulators x2 = 8 banks exactly.  Scores
    only wait on exp three pairs back; ctx matmuls trail scores by two
    pairs (deeper trailing measured worse).
  * exp split 5:3 scalar:DVE (bf16 Schraudolph int16 bit trick on DVE),
    interleaved s,s,d,s,d,s,s,d so the scalar never builds a backlog.
    Every softmax row sees a uniform 3/8 trick share (rel err 1.03e-2
    vs the 2e-2 gate; all-fp8/DoubleRow variants measured 2-5e-2 in a
    host-side quantization study -- attention output here is a
    near-uniform average, std ~1/45 of v, so e4m3 noise lands ~1:1 on
    the output; fp8 is unusable everywhere in this problem).
  * Softmax denominator: ones column FIRST in v' (padded to 128 cols:
    [ones, 63 zeros, v0..63]) so the denominator lands on psum
    PARTITION 0 -- the custom-DVE reciprocal_approx_fast and the
    gpsimd partition_broadcast ucode both silently require base
    partition 0 on real HW (verified; partition-64 inputs return
    garbage).  Chain per iteration: DVE row copy + reciprocal, gpsimd
    broadcast to all 128 partitions, then two DVE multiplies (psum rows
    64:128, a legal DVE base-64 window) + SBUF->SBUF partition-shift
    DMAs into ctx_sb.  Nothing here touches the PE or the score pool,
    and the chain is short enough that the multiplies never wait (a
    late rec head-of-line-blocks the DVE exp queue -- measured 2us
    stalls with the earlier DMA-shift chain).
  * HW gotchas baked in: two CONCURRENT row-tiled matmuls (the K=64
    head pair at tile rows 0/64) must write DIFFERENT psum banks or
    the device hangs; one psum accumulation group per 2KB bank
    (zero-region granularity); each dma_start costs ~3us of queue
    time, so stage-1 inputs ship as a few fat DMAs (x quarter in one,
    weights f-chunk-major so matmul #1 only waits for x + 1/8 of wqk).
  * Stage 1 f32r QKV matmuls from bf16-bit inputs; psum pairs
    [128,1024] so each psum->bf16 cast covers two matmul outputs,
    alternating scalar/DVE.  Stage 3 bf16 with one 2-bank psum per
    token block, consecutive matmuls sharing their stationary ctx
    chunk, single [128,1024] cast, one output DMA per block.
Host sums the two partials per batch and adds the bias.
"""

import numpy as np

import concourse.bass as bass
import concourse.tile as tile
from concourse import bacc, library_config, mybir
from concourse import bass_utils

f32 = mybir.dt.float32
f32r = mybir.dt.float32r
f8 = mybir.dt.float8e4
bf16 = mybir.dt.bfloat16
i16 = mybir.dt.int16
u16 = mybir.dt.uint16
Exp = mybir.ActivationFunctionType.Exp
Copy = mybir.ActivationFunctionType.Copy
Mult = mybir.AluOpType.mult
Add = mybir.AluOpType.add

TOK = 2048          # tokens per batch
DM = 1024           # model dim
DL = 512            # local q/k/v feature dim (8 heads x 64)
D = 64              # head dim
NH = 8              # local heads
NPAIR = 4           # head pairs (partition blocks of ctx/qk)
KT = 8              # dm / 128 contraction tiles
SCALE = DM ** (-0.5)
N_CORES = 8

# bf16 Schraudolph: bf16bits(exp(s*SCALE)) ~= trunc(s * A16 + B16).
# The -6.75 zeroes the trick's mean multiplicative bias (so scalar-exp'd
# and trick'd key-chunks weight consistently inside one softmax row);
# it splits the difference between truncating (-6.5) and rounding (-7.0)
# f32->i16 conversion.
A16 = 128.0 * SCALE / float(np.log(2.0))
B16 = 127.0 * 128.0 - 6.75

# exp engine schedule, cycled per PAIRED exp tile ([128,1024] covering
# TWO key-chunks x both heads at 256-query granularity): 5/8 scalar,
# 3/8 DVE, which balances the engines (DVE also carries the softmax
# division) and keeps every softmax row's Schraudolph share a uniform
# 3/8.  (gpsimd cannot access PSUM on TRN2, so it cannot help w/ exp.)
EXP_SCHED = ("s", "s", "d", "s", "d", "s", "s", "d")


def _build(tc, xT, wqkT, wvT, wo16, out_p):
    nc = tc.nc
    # gpsimd ucode library with partition_broadcast (softmax denominator)
    nc.gpsimd.load_library(library_config.attn)

    w3p = tc.alloc_tile_pool(name="w3", bufs=1)
    ctp = tc.alloc_tile_pool(name="ctp", bufs=1)
    qkp = tc.alloc_tile_pool(name="qkp", bufs=1)
    vp_ = tc.alloc_tile_pool(name="vp", bufs=1)

    ctx_sb = ctp.tile([128, NPAIR, TOK], bf16, tag="ctx")    # 16 KB/part
    qk_sb = qkp.tile([128, 2 * NPAIR, TOK], bf16, tag="qk")  # 32 KB/part
    # v' columns: [ones, 63 zero-pad, v0..v63] = 128.  The ones column in
    # position 0 puts the softmax denominator on psum PARTITION 0 (legal
    # base for the custom-DVE reciprocal and the gpsimd broadcast, which
    # silently require it on HW) while the ctx rows land at 64:128 (legal
    # DVE base-64 64-partition window for the division multiplies).
    v_sb = vp_.tile([128, 16, NH, 128], bf16, tag="v")       # 32 KB/part
    # output-projection weights prefetched up front so stage 3 never
    # waits on DMA
    wout_sb = w3p.tile([128, NPAIR, DM], u16, tag="wout")    # 8 KB/part
    nc.sync.dma_start(wout_sb[:], wo16[:])

    # ones column FIRST in v' (softmax denominator accumulator): the
    # denominator then lands on psum PARTITION 0, where the custom-DVE
    # reciprocal and the gpsimd partition_broadcast actually work on HW
    # (both silently require base partition 0) -- no DMA shift needed
    # on the latency-critical reciprocal chain
    nc.vector.memset(v_sb[:, :, :, 0:1], 1.0)
    nc.vector.memset(v_sb[:, :, :, 1:64], 0.0)

    # ---- stage 1: QKV projection in f32r, token-quarters of 512 ----
    # psum tiles are PAIRED [128,1024] (2 banks) so every psum->bf16 cast
    # covers two matmul outputs: the ~220ns per-instruction engine
    # overhead is paid half as often on both DVE and scalar.
    with (
        tc.tile_pool(name="w1", bufs=1) as wp,
        tc.tile_pool(name="x", bufs=2) as xp,
        tc.tile_pool(name="psq", bufs=2, space="PSUM") as psq,
    ):
        # wqkT is staged f-chunk-major [128, 8(f), KT, 128] so the first
        # q/k matmul group only waits for x plus 1/8 of the weights
        wqk_sb = wp.tile([128, 8, KT, 128], u16, tag="wqk")   # 16 KB/part
        wv_sb = wp.tile([128, KT, DL], u16, tag="wv")         # 8 KB/part
        xt0 = xp.tile([128, KT, 512], u16, tag="x")
        # few, fat DMAs: each dma_start costs ~3us of queue time here, so
        # batch everything except the f0 weight chunk (which gates MM #1)
        nc.sync.dma_start(xt0[:], xT[:, :, 0:512])
        nc.sync.dma_start(wqk_sb[:, 0], wqkT[:, 0])
        nc.sync.dma_start(wqk_sb[:, 1:8], wqkT[:, 1:8])
        nc.sync.dma_start(wv_sb[:], wvT[:])
        ncast = 0
        for tq in range(4):
            ts512 = slice(tq * 512, (tq + 1) * 512)
            if tq == 0:
                xt = xt0
            else:
                xt = xp.tile([128, KT, 512], u16, tag="x")
                nc.sync.dma_start(xt[:], xT[:, :, ts512])
            # q/k (transposed layout): out [feat 128, tok 512], f-pairs
            for fp in range(4):
                ps = psq.tile([128, 1024], f32, tag="qk", bufs=3)
                for half in range(2):
                    f = 2 * fp + half
                    hs = slice(half * 512, (half + 1) * 512)
                    for kt in range(KT):
                        nc.tensor.matmul(
                            ps[:, hs],
                            wqk_sb[:, f, kt, :].bitcast(bf16),
                            xt[:, kt, :].bitcast(bf16),
                            start=(kt == 0), stop=(kt == KT - 1))
                with nc.allow_low_precision(reason="bf16"):
                    dst = qk_sb[:, 2 * fp:2 * fp + 2, ts512]
                    src = ps[:].rearrange("p (two t) -> p two t", two=2)
                    if ncast % 2 == 0:
                        nc.vector.tensor_copy(dst, src)
                    else:
                        nc.scalar.activation(dst, src, Copy)
                ncast += 1
            # v (natural layout): out [tok 128, feat 512], tt-pairs
            for tp in range(2):
                ps = psq.tile([128, 1024], f32, tag="v", bufs=1)
                for half in range(2):
                    tt = 2 * tp + half
                    hs = slice(half * 512, (half + 1) * 512)
                    for kt in range(KT):
                        nc.tensor.matmul(
                            ps[:, hs],
                            xt[:, kt, tt * 128:(tt + 1) * 128].bitcast(bf16),
                            wv_sb[:, kt, :].bitcast(bf16),
                            start=(kt == 0), stop=(kt == KT - 1))
                jt0 = tq * 4 + 2 * tp
                with nc.allow_low_precision(reason="bf16 v"):
                    nc.vector.tensor_copy(
                        v_sb[:, jt0:jt0 + 2, :, 64:128],
                        ps[:].rearrange("p (two h d) -> p two h d",
                                        two=2, h=NH))

    # ---- stage 2: attention, per head-pair, query-eighths of 256 ----
    with (
        tc.tile_pool(name="psS", bufs=3, space="PSUM") as psS,
        tc.tile_pool(name="psC", bufs=2, space="PSUM") as psC,
        tc.tile_pool(name="ep", bufs=8) as ep,
        tc.tile_pool(name="dv", bufs=2) as dv,
    ):
        eidx = 0

        def div_tail(ps_c, rec, ihs, p):
            # softmax division for both heads of the pair.  rec (rows
            # 64:128 = broadcast reciprocal denominators) produced at
            # the end of the accumulating iteration with a short chain
            # (DVE copy + DVE recip + gpsimd broadcast, all at their
            # native partitions), so these multiplies never wait and
            # never head-of-line-block the DVE's exp queue.  ctx lives
            # on psum rows 64:128; both heads shift partitions via cheap
            # off-critical-path SBUF->SBUF DMAs.
            with nc.allow_low_precision(reason="bf16 ctx"):
                for hh in (0, 1):
                    tmp = dv.tile([128, 256], bf16, tag="tmp",
                                  name=f"tmp{hh}")
                    nc.vector.tensor_tensor(
                        out=tmp[64:128, :],
                        in0=ps_c[64:128, hh * 256:(hh + 1) * 256],
                        in1=rec[64:128, hh * 256:(hh + 1) * 256], op=Mult)
                    nc.sync.dma_start(
                        ctx_sb[64 * hh:64 * hh + 64, p, ihs], tmp[64:128, :])

        pending_div = None   # previous iteration's division, deferred to
        # the middle of the next iteration (all off the PE critical path)
        # 256-query iterations: score psum per key-chunk is ONE bank
        # ([128, 2 heads x 256q]) so a [128,1024] 2-bank tile covers TWO
        # key-chunks -> exp stays a paired instruction AND the score pool
        # holds 3 slots: scores(jp) only wait on exp(jp-3), deep enough
        # that neither exp latency nor engine jitter ever stalls the PE.
        for p in range(4):
            for i8 in range(8):
                ihs = slice(i8 * 256, (i8 + 1) * 256)
                # both heads' ctx accumulators in one 1-bank psum tile
                ps_c = psC.tile([128, 512], f32, tag="C")

                def ctx_mm(jp_c, e_kc):
                    # ONE psum accumulation group for the whole bank (the
                    # zero-region granularity is a 2KB bank): start on the
                    # very first matmul, stop on the very last.  The ctx
                    # matmuls use full-height lhsT so they never run
                    # concurrently -> same-bank writes are safe here.
                    for sub in (0, 1):
                        kc = 2 * jp_c + sub
                        for hh in (0, 1):
                            nc.tensor.matmul(
                                ps_c[:, hh * 256:(hh + 1) * 256],
                                v_sb[:, kc, 2 * p + hh, :],
                                e_kc[:, hh * 512 + sub * 256:
                                     hh * 512 + (sub + 1) * 256],
                                start=(kc == 0 and hh == 0),
                                stop=(kc == 15 and hh == 1))

                e_hist = []   # E tiles whose ctx matmuls are still pending
                for jp in range(8):
                    ps_s = psS.tile([128, 1024], f32, tag="S")
                    # bank 0 = head a (sub0|sub1 columns), bank 1 = head b:
                    # the two heads' row-tiled matmuls run CONCURRENTLY in
                    # the PE, so they must write different psum banks (a
                    # same-bank concurrent write hangs the device); the
                    # same-head/same-bank sub matmuls are one group each
                    for sub in (0, 1):
                        jt = 2 * jp + sub
                        js = slice(jt * 128, (jt + 1) * 128)
                        for poff, hh in ((0, 0), (64, 1)):
                            nc.tensor.matmul(
                                ps_s[:, hh * 512 + sub * 256:
                                     hh * 512 + (sub + 1) * 256],
                                qk_sb[poff:poff + D, 4 + p, js],
                                qk_sb[poff:poff + D, p, ihs],
                                start=(sub == 0), stop=(sub == 1))
                    if jp >= 2:
                        ctx_mm(jp - 2, e_hist.pop(0))
                    if jp == 4 and pending_div is not None:
                        div_tail(*pending_div)
                        pending_div = None
                    e_t = ep.tile([128, 1024], bf16, tag="E")
                    eng = EXP_SCHED[eidx % len(EXP_SCHED)]
                    eidx += 1
                    with nc.allow_low_precision(reason="exp"):
                        if eng == "s":
                            nc.scalar.activation(e_t[:], ps_s[:], Exp,
                                                 scale=SCALE)
                        else:
                            nc.vector.tensor_scalar(
                                e_t[:].bitcast(i16), ps_s[:],
                                A16, B16, Mult, Add)
                    e_hist.append(e_t)
                # drain the last two key-chunk pairs' ctx
                ctx_mm(6, e_hist[0])
                ctx_mm(7, e_hist[1])
                # denominators sit on psum ROW 0 (ones column is first in
                # v'): copy out, reciprocal, and broadcast all operate at
                # base partition 0 natively -- short low-latency chain
                den0 = dv.tile([1, 512], f32, tag="den0")
                nc.vector.tensor_copy(den0[:], ps_c[0:1, :])
                rec0 = dv.tile([1, 512], f32, tag="rec0")
                nc.vector.reciprocal_approx_fast(rec0[:], den0[:])
                rec = dv.tile([128, 512], f32, tag="rec")
                nc.gpsimd.partition_broadcast(rec[:], rec0[:], channels=128)
                pending_div = (ps_c, rec, ihs, p)
        div_tail(*pending_div)

    vp_.release()
    qkp.release()

    # ---- stage 3: output projection (bf16) ----
    # one 2-bank psum per token-block; consecutive matmuls share their
    # stationary ctx chunk, and the psum->f32 cast is a single [128,1024]
    # instruction alternating between scalar and DVE
    with (
        tc.tile_pool(name="psO", bufs=2, space="PSUM") as psO,
        tc.tile_pool(name="ot", bufs=3) as otp,
    ):
        for tt in range(16):
            o_t = otp.tile([128, DM], f32, tag="o")
            ps = psO.tile([128, 1024], f32, tag="O")
            for pb in range(NPAIR):
                for fc in range(2):
                    fs = slice(fc * 512, (fc + 1) * 512)
                    nc.tensor.matmul(
                        ps[:, fs], ctx_sb[:, pb, tt * 128:(tt + 1) * 128],
                        wout_sb[:, pb, fs].bitcast(bf16),
                        start=(pb == 0), stop=(pb == NPAIR - 1))
            if tt % 2 == 0:
                nc.scalar.activation(o_t[:], ps[:], Copy)
            else:
                nc.vector.tensor_copy(o_t[:], ps[:])
            nc.sync.dma_start(out_p[tt * 128:(tt + 1) * 128, :], o_t[:])
    ctp.release()
    w3p.release()


_CACHE = {}


def _get_nc():
    if "nc" not in _CACHE:
        nc = bacc.Bacc("TRN2", target_bir_lowering=False, debug=False)
        xT = nc.dram_tensor("xT", [128, KT, TOK], u16, kind="ExternalInput").ap()
        wqkT = nc.dram_tensor("wqkT", [128, 8, KT, 128], u16,
                              kind="ExternalInput").ap()
        wvT = nc.dram_tensor("wvT", [128, KT, DL], u16,
                             kind="ExternalInput").ap()
        wo16 = nc.dram_tensor("wo16", [128, NPAIR, DM], u16,
                              kind="ExternalInput").ap()
        out_p = nc.dram_tensor("out_p", [TOK, DM], f32, kind="ExternalOutput").ap()
        with tile.TileContext(nc) as tc:
            _build(tc, xT, wqkT, wvT, wo16, out_p)
        nc.compile()
        _CACHE["nc"] = nc
    return _CACHE["nc"]


def _bf16_bits(x):
    b = np.ascontiguousarray(np.asarray(x, np.float32)).view(np.uint32)
    return ((b + 0x7FFF + ((b >> 16) & 1)) >> 16).astype(np.uint16)


def _fold(a):
    """[DM, cols] -> [128, KT, cols] bf16 bits, partition-major dm chunks."""
    b = _bf16_bits(a)
    return np.ascontiguousarray(
        b.reshape(KT, 128, a.shape[1]).transpose(1, 0, 2))


def make_in_maps(x, w_qkv, w_out):
    in_maps = []
    xTb = {b: _fold(np.ascontiguousarray(x[b].T)) for b in range(4)}
    for c in range(N_CORES):
        b, g = c // 2, c % 2
        gs = slice(g * DL, (g + 1) * DL)
        wq = w_qkv[0 * DM + g * DL:0 * DM + (g + 1) * DL]
        wk = w_qkv[1 * DM + g * DL:1 * DM + (g + 1) * DL]
        wv = w_qkv[2 * DM + g * DL:2 * DM + (g + 1) * DL]
        woT = np.ascontiguousarray(w_out[:, gs].T)        # [DL, DM]
        wo16 = np.ascontiguousarray(
            _bf16_bits(woT).reshape(NPAIR, 128, DM).transpose(1, 0, 2))
        wqk_fold = _fold(np.ascontiguousarray(np.concatenate([wq, wk], 0).T))
        # [128, KT, 1024] -> f-chunk-major [128, 8, KT, 128]
        wqk_fold = np.ascontiguousarray(
            wqk_fold.reshape(128, KT, 8, 128).transpose(0, 2, 1, 3))
        in_maps.append({
            "xT": xTb[b],
            "wqkT": wqk_fold,
            "wvT": _fold(np.ascontiguousarray(wv.T)),
            "wo16": wo16,
        })
    return in_maps


def kernel(x, w_qkv, w_out, b_out, _trace=False):
    x = np.asarray(x, dtype=np.float32)
    w_qkv = np.asarray(w_qkv, dtype=np.float32)
    w_out = np.asarray(w_out, dtype=np.float32)
    b_out = np.asarray(b_out, dtype=np.float32)

    nc = _get_nc()
    in_maps = make_in_maps(x, w_qkv, w_out)
    res = bass_utils.run_bass_kernel_spmd(
        nc, in_maps, core_ids=list(range(N_CORES)), trace=_trace)
    out = np.empty((4, TOK, DM), dtype=np.float32)
    for b in range(4):
        out[b] = res.results[2 * b]["out_p"] + res.results[2 * b + 1]["out_p"]
    out += b_out
    if _trace:
        kernel.last_results = res
    return out



# revision 64
# speedup vs baseline: 1.0525x; 1.0323x over previous
"""Multi-head attention (b=4, n=2048, dm=1024, h=16) on 8 TRN2 NeuronCores.

Sharding: batch (4) x head-group (2) -> 8 cores, Megatron-style.
Core c handles batch c//2 and heads [8*(c%2), 8*(c%2)+8).

v5 design (HW 380us; v3 was 440us, v1 616us).  Trace-driven findings:
  * Stage 2 is jointly limited by PE matmul streaming (~225us of
    N-cycle work), exp throughput (33.5M elements across scalar+DVE),
    and the scores->exp->scores PSUM-slot dependency ring.  v3 ran the
    ring with 2 slots and 4:1 exp split: warm clock but exp-bound
    (~300us).  v5 uses 256-query iterations so a score psum bank holds
    [128, 2 heads x 256q] and a [128,1024] 2-bank tile covers TWO
    key-chunks: exp stays a big paired instruction (engine overhead
    ~220ns amortized over 1024 cols) AND the score pool holds 3 slots
    (6 banks) + 1-bank ctx accumulators x2 = 8 banks exactly.  Scores
    only wait on exp three pairs back; ctx matmuls trail scores by two
    pairs (deeper trailing measured worse).
  * exp split 5:3 scalar:DVE (bf16 Schraudolph int16 bit trick on DVE),
    interleaved s,s,d,s,d,s,s,d so the scalar never builds a backlog.
    Every softmax row sees a uniform 3/8 trick share (rel err 1.03e-2
    vs the 2e-2 gate; all-fp8/DoubleRow variants measured 2-5e-2 in a
    host-side quantization study -- attention output here is a
    near-uniform average, std ~1/45 of v, so e4m3 noise lands ~1:1 on
    the output; fp8 is unusable everywhere in this problem).
  * Softmax denominator: ones column FIRST in v' (padded to 128 cols:
    [ones, 63 zeros, v0..63]) so the denominator lands on psum
    PARTITION 0 -- the custom-DVE reciprocal_approx_fast and the
    gpsimd partition_broadcast ucode both silently require base
    partition 0 on real HW (verified; partition-64 inputs return
    garbage).  Chain per iteration: DVE row copy + reciprocal, gpsimd
    broadcast to all 128 partitions, then two DVE multiplies (psum rows
    64:128, a legal DVE base-64 window) + SBUF->SBUF partition-shift
    DMAs into ctx_sb.  Nothing here touches the PE or the score pool,
    and the chain is short enough that the multiplies never wait (a
    late rec head-of-line-blocks the DVE exp queue -- measured 2us
    stalls with the earlier DMA-shift chain).
  * HW gotchas baked in: two CONCURRENT row-tiled matmuls (the K=64
    head pair at tile rows 0/64) must write DIFFERENT psum banks or
    the device hangs; one psum accumulation group per 2KB bank
    (zero-region granularity); each dma_start costs ~3us of queue
    time, so stage-1 inputs ship as a few fat DMAs (x quarter in one,
    weights f-chunk-major so matmul #1 only waits for x + 1/8 of wqk).
  * Stage 1 f32r QKV matmuls from bf16-bit inputs; psum pairs
    [128,1024] so each psum->bf16 cast covers two matmul outputs,
    alternating scalar/DVE.  Stage 3 bf16 with one 2-bank psum per
    token block, consecutive matmuls sharing their stationary ctx
    chunk, single [128,1024] cast, one output DMA per block.
Host sums the two partials per batch and adds the bias.
"""

import numpy as np

import concourse.bass as bass
import concourse.tile as tile
from concourse import bacc, library_config, mybir
from concourse import bass_utils

f32 = mybir.dt.float32
f32r = mybir.dt.float32r
f8 = mybir.dt.float8e4
bf16 = mybir.dt.bfloat16
i16 = mybir.dt.int16
u16 = mybir.dt.uint16
Exp = mybir.ActivationFunctionType.Exp
Copy = mybir.ActivationFunctionType.Copy
Mult = mybir.AluOpType.mult
Add = mybir.AluOpType.add

TOK = 2048          # tokens per batch
DM = 1024           # model dim
DL = 512            # local q/k/v feature dim (8 heads x 64)
D = 64              # head dim
NH = 8              # local heads
NPAIR = 4           # head pairs (partition blocks of ctx/qk)
KT = 8              # dm / 128 contraction tiles
SCALE = DM ** (-0.5)
N_CORES = 8

# bf16 Schraudolph: bf16bits(exp(s*SCALE)) ~= trunc(s * A16 + B16).
# The -6.75 zeroes the trick's mean multiplicative bias (so scalar-exp'd
# and trick'd key-chunks weight consistently inside one softmax row);
# it splits the difference between truncating (-6.5) and rounding (-7.0)
# f32->i16 conversion.
A16 = 128.0 * SCALE / float(np.log(2.0))
B16 = 127.0 * 128.0 - 6.75

# exp engine schedule, cycled per PAIRED exp tile ([128,1024] covering
# TWO key-chunks x both heads at 256-query granularity): 5/8 scalar,
# 3/8 DVE, which balances the engines (DVE also carries the softmax
# division) and keeps every softmax row's Schraudolph share a uniform
# 3/8.  (gpsimd cannot access PSUM on TRN2, so it cannot help w/ exp.)
EXP_SCHED = ("s", "s", "d", "s", "d", "s", "s", "d")


def _build(tc, xT, wqkT, wvT, wo16, out_p):
    nc = tc.nc
    # gpsimd ucode library with partition_broadcast (softmax denominator)
    nc.gpsimd.load_library(library_config.attn)

    w3p = tc.alloc_tile_pool(name="w3", bufs=1)
    ctp = tc.alloc_tile_pool(name="ctp", bufs=1)
    qkp = tc.alloc_tile_pool(name="qkp", bufs=1)
    vp_ = tc.alloc_tile_pool(name="vp", bufs=1)

    ctx_sb = ctp.tile([128, NPAIR, TOK], bf16, tag="ctx")    # 16 KB/part
    qk_sb = qkp.tile([128, 2 * NPAIR, TOK], bf16, tag="qk")  # 32 KB/part
    # v' columns: [ones, 63 zero-pad, v0..v63] = 128.  The ones column in
    # position 0 puts the softmax denominator on psum PARTITION 0 (legal
    # base for the custom-DVE reciprocal and the gpsimd broadcast, which
    # silently require it on HW) while the ctx rows land at 64:128 (legal
    # DVE base-64 64-partition window for the division multiplies).
    v_sb = vp_.tile([128, 16, NH, 128], bf16, tag="v")       # 32 KB/part
    # output-projection weights prefetched up front so stage 3 never
    # waits on DMA
    wout_sb = w3p.tile([128, NPAIR, DM], u16, tag="wout")    # 8 KB/part
    nc.sync.dma_start(wout_sb[:], wo16[:])

    # ones column FIRST in v' (softmax denominator accumulator): the
    # denominator then lands on psum PARTITION 0, where the custom-DVE
    # reciprocal and the gpsimd partition_broadcast actually work on HW
    # (both silently require base partition 0) -- no DMA shift needed
    # on the latency-critical reciprocal chain
    nc.vector.memset(v_sb[:, :, :, 0:1], 1.0)
    nc.vector.memset(v_sb[:, :, :, 1:64], 0.0)

    # ---- stage 1: QKV projection in f32r, token-quarters of 512 ----
    # psum tiles are PAIRED [128,1024] (2 banks) so every psum->bf16 cast
    # covers two matmul outputs: the ~220ns per-instruction engine
    # overhead is paid half as often on both DVE and scalar.
    with (
        tc.tile_pool(name="w1", bufs=1) as wp,
        tc.tile_pool(name="x", bufs=2) as xp,
        tc.tile_pool(name="psq", bufs=2, space="PSUM") as psq,
    ):
        # wqkT is staged f-chunk-major [128, 8(f), KT, 128] so the first
        # q/k matmul group only waits for x plus 1/8 of the weights
        wqk_sb = wp.tile([128, 8, KT, 128], u16, tag="wqk")   # 16 KB/part
        wv_sb = wp.tile([128, KT, DL], u16, tag="wv")         # 8 KB/part
        xt0 = xp.tile([128, KT, 512], u16, tag="x")
        # few, fat DMAs: each dma_start costs ~3us of queue time here, so
        # batch everything except the f0 weight chunk (which gates MM #1)
        nc.sync.dma_start(xt0[:], xT[:, :, 0:512])
        nc.sync.dma_start(wqk_sb[:, 0], wqkT[:, 0])
        nc.sync.dma_start(wqk_sb[:, 1:8], wqkT[:, 1:8])
        nc.sync.dma_start(wv_sb[:], wvT[:])
        ncast = 0
        for tq in range(4):
            ts512 = slice(tq * 512, (tq + 1) * 512)
            if tq == 0:
                xt = xt0
            else:
                xt = xp.tile([128, KT, 512], u16, tag="x")
                nc.sync.dma_start(xt[:], xT[:, :, ts512])
            # q/k (transposed layout): out [feat 128, tok 512], f-pairs
            for fp in range(4):
                ps = psq.tile([128, 1024], f32, tag="qk", bufs=3)
                for half in range(2):
                    f = 2 * fp + half
                    hs = slice(half * 512, (half + 1) * 512)
                    for kt in range(KT):
                        nc.tensor.matmul(
                            ps[:, hs],
                            wqk_sb[:, f, kt, :].bitcast(bf16),
                            xt[:, kt, :].bitcast(bf16),
                            start=(kt == 0), stop=(kt == KT - 1))
                with nc.allow_low_precision(reason="bf16"):
                    dst = qk_sb[:, 2 * fp:2 * fp + 2, ts512]
                    src = ps[:].rearrange("p (two t) -> p two t", two=2)
                    if ncast % 2 == 0:
                        nc.vector.tensor_copy(dst, src)
                    else:
                        nc.scalar.activation(dst, src, Copy)
                ncast += 1
            # v (natural layout): out [tok 128, feat 512], tt-pairs
            for tp in range(2):
                ps = psq.tile([128, 1024], f32, tag="v", bufs=1)
                for half in range(2):
                    tt = 2 * tp + half
                    hs = slice(half * 512, (half + 1) * 512)
                    for kt in range(KT):
                        nc.tensor.matmul(
                            ps[:, hs],
                            xt[:, kt, tt * 128:(tt + 1) * 128].bitcast(bf16),
                            wv_sb[:, kt, :].bitcast(bf16),
                            start=(kt == 0), stop=(kt == KT - 1))
                jt0 = tq * 4 + 2 * tp
                with nc.allow_low_precision(reason="bf16 v"):
                    nc.vector.tensor_copy(
                        v_sb[:, jt0:jt0 + 2, :, 64:128],
                        ps[:].rearrange("p (two h d) -> p two h d",
                                        two=2, h=NH))

    # ---- stage 2: attention, per head-pair, query-eighths of 256 ----
    with (
        tc.tile_pool(name="psS", bufs=3, space="PSUM") as psS,
        tc.tile_pool(name="psC", bufs=2, space="PSUM") as psC,
        tc.tile_pool(name="ep", bufs=8) as ep,
        tc.tile_pool(name="dv", bufs=2) as dv,
    ):
        eidx = 0

        def div_tail(ps_c, rec, ihs, p):
            # softmax division for both heads of the pair.  rec (rows
            # 64:128 = broadcast reciprocal denominators) produced at
            # the end of the accumulating iteration with a short chain
            # (DVE copy + DVE recip + gpsimd broadcast, all at their
            # native partitions), so these multiplies never wait and
            # never head-of-line-block the DVE's exp queue.  ctx lives
            # on psum rows 64:128; both heads shift partitions via cheap
            # off-critical-path SBUF->SBUF DMAs.
            with nc.allow_low_precision(reason="bf16 ctx"):
                for hh in (0, 1):
                    tmp = dv.tile([128, 256], bf16, tag="tmp",
                                  name=f"tmp{hh}")
                    nc.vector.tensor_tensor(
                        out=tmp[64:128, :],
                        in0=ps_c[64:128, hh * 256:(hh + 1) * 256],
                        in1=rec[64:128, hh * 256:(hh + 1) * 256], op=Mult)
                    nc.sync.dma_start(
                        ctx_sb[64 * hh:64 * hh + 64, p, ihs], tmp[64:128, :])

        pending_div = None   # (ps_c, rec, ihs, p) awaiting division
        hist = []            # (ps_c, p, ihs, jp, e_t) with pending ctx

        def ctx_mm(ps_c_t, p_t, jp_c, e_kc):
            # ONE psum accumulation group for the whole bank (zero-region
            # granularity is a 2KB bank): start on the very first matmul,
            # stop on the very last.  Full-height lhsT -> the ctx matmuls
            # never run concurrently, so same-bank writes are safe.
            for sub in (0, 1):
                kc = 2 * jp_c + sub
                for hh in (0, 1):
                    nc.tensor.matmul(
                        ps_c_t[:, hh * 256:(hh + 1) * 256],
                        v_sb[:, kc, 2 * p_t + hh, :],
                        e_kc[:, hh * 512 + sub * 256:
                             hh * 512 + (sub + 1) * 256],
                        start=(kc == 0 and hh == 0),
                        stop=(kc == 15 and hh == 1))

        def den_chain(ps_c_t, p_t, ihs_t):
            # denominators sit on psum ROW 0 (ones column is first in
            # v'): copy out, reciprocal, and broadcast all operate at
            # base partition 0 natively -- short low-latency chain
            den0 = dv.tile([1, 512], f32, tag="den0")
            nc.vector.tensor_copy(den0[:], ps_c_t[0:1, :])
            rec0 = dv.tile([1, 512], f32, tag="rec0")
            nc.vector.reciprocal_approx_fast(rec0[:], den0[:])
            rec = dv.tile([128, 512], f32, tag="rec")
            nc.gpsimd.partition_broadcast(rec[:], rec0[:], channels=128)
            return (ps_c_t, rec, ihs_t, p_t)

        # one flat 256-pair software pipeline over (p, i8, jp): the two
        # trailing ctx matmul groups of each (p, i8) iteration wrap into
        # the next iteration instead of draining at its end, so the PE
        # never sits through the exp tail at an iteration boundary
        ps_c = None
        for g in range(256):
            p, i8, jp = g // 64, (g // 8) % 8, g % 8
            ihs = slice(i8 * 256, (i8 + 1) * 256)
            if jp == 0:
                # both heads' ctx accumulators in one 1-bank psum tile
                ps_c = psC.tile([128, 512], f32, tag="C")
            # bank 0 = head a (sub0|sub1 columns), bank 1 = head b: the
            # two heads' row-tiled matmuls run CONCURRENTLY in the PE,
            # so they must write different psum banks (a same-bank
            # concurrent write hangs the device); the same-head/same-
            # bank sub matmuls form one group each
            ps_s = psS.tile([128, 1024], f32, tag="S")
            for sub in (0, 1):
                jt = 2 * jp + sub
                js = slice(jt * 128, (jt + 1) * 128)
                for poff, hh in ((0, 0), (64, 1)):
                    nc.tensor.matmul(
                        ps_s[:, hh * 512 + sub * 256:
                             hh * 512 + (sub + 1) * 256],
                        qk_sb[poff:poff + D, 4 + p, js],
                        qk_sb[poff:poff + D, p, ihs],
                        start=(sub == 0), stop=(sub == 1))
            if len(hist) == 2:
                cps, cp, cihs, cjp, ce = hist.pop(0)
                ctx_mm(cps, cp, cjp, ce)
                if cjp == 7:
                    # that iteration's ctx psum is now complete: launch
                    # its denominator/reciprocal chain immediately
                    pending_div = den_chain(cps, cp, cihs)
            if jp == 5 and pending_div is not None:
                div_tail(*pending_div)
                pending_div = None
            e_t = ep.tile([128, 1024], bf16, tag="E")
            eng = EXP_SCHED[eidx % len(EXP_SCHED)]
            eidx += 1
            with nc.allow_low_precision(reason="exp"):
                if eng == "s":
                    nc.scalar.activation(e_t[:], ps_s[:], Exp,
                                         scale=SCALE)
                else:
                    nc.vector.tensor_scalar(
                        e_t[:].bitcast(i16), ps_s[:],
                        A16, B16, Mult, Add)
            hist.append((ps_c, p, ihs, jp, e_t))
        # final drain: last two pairs' ctx, then the last den chain
        for cps, cp, cihs, cjp, ce in hist:
            ctx_mm(cps, cp, cjp, ce)
        pending_div = den_chain(ps_c, 3, slice(7 * 256, 8 * 256))
        div_tail(*pending_div)

    vp_.release()
    qkp.release()

    # ---- stage 3: output projection (bf16) ----
    # one 2-bank psum per token-block; consecutive matmuls share their
    # stationary ctx chunk, and the psum->f32 cast is a single [128,1024]
    # instruction alternating between scalar and DVE
    with (
        tc.tile_pool(name="psO", bufs=2, space="PSUM") as psO,
        tc.tile_pool(name="ot", bufs=3) as otp,
    ):
        for tt in range(16):
            o_t = otp.tile([128, DM], f32, tag="o")
            ps = psO.tile([128, 1024], f32, tag="O")
            for pb in range(NPAIR):
                for fc in range(2):
                    fs = slice(fc * 512, (fc + 1) * 512)
                    nc.tensor.matmul(
                        ps[:, fs], ctx_sb[:, pb, tt * 128:(tt + 1) * 128],
                        wout_sb[:, pb, fs].bitcast(bf16),
                        start=(pb == 0), stop=(pb == NPAIR - 1))
            if tt % 2 == 0:
                nc.scalar.activation(o_t[:], ps[:], Copy)
            else:
                nc.vector.tensor_copy(o_t[:], ps[:])
            nc.sync.dma_start(out_p[tt * 128:(tt + 1) * 128, :], o_t[:])
    ctp.release()
    w3p.release()


_CACHE = {}


def _get_nc():
    if "nc" not in _CACHE:
        nc = bacc.Bacc("TRN2", target_bir_lowering=False, debug=False)
        xT = nc.dram_tensor("xT", [128, KT, TOK], u16, kind="ExternalInput").ap()
        wqkT = nc.dram_tensor("wqkT", [128, 8, KT, 128], u16,
                              kind="ExternalInput").ap()
        wvT = nc.dram_tensor("wvT", [128, KT, DL], u16,
                             kind="ExternalInput").ap()
        wo16 = nc.dram_tensor("wo16", [128, NPAIR, DM], u16,
                              kind="ExternalInput").ap()
        out_p = nc.dram_tensor("out_p", [TOK, DM], f32, kind="ExternalOutput").ap()
        with tile.TileContext(nc) as tc:
            _build(tc, xT, wqkT, wvT, wo16, out_p)
        nc.compile()
        _CACHE["nc"] = nc
    return _CACHE["nc"]


def _bf16_bits(x):
    b = np.ascontiguousarray(np.asarray(x, np.float32)).view(np.uint32)
    return ((b + 0x7FFF + ((b >> 16) & 1)) >> 16).astype(np.uint16)


def _fold(a):
    """[DM, cols] -> [128, KT, cols] bf16 bits, partition-major dm chunks."""
    b = _bf16_bits(a)
    return np.ascontiguousarray(
        b.reshape(KT, 128, a.shape[1]).transpose(1, 0, 2))


def make_in_maps(x, w_qkv, w_out):
    in_maps = []
    xTb = {b: _fold(np.ascontiguousarray(x[b].T)) for b in range(4)}
    for c in range(N_CORES):
        b, g = c // 2, c % 2
        gs = slice(g * DL, (g + 1) * DL)
        wq = w_qkv[0 * DM + g * DL:0 * DM + (g + 1) * DL]
        wk = w_qkv[1 * DM + g * DL:1 * DM + (g + 1) * DL]
        wv = w_qkv[2 * DM + g * DL:2 * DM + (g + 1) * DL]
        woT = np.ascontiguousarray(w_out[:, gs].T)        # [DL, DM]
        wo16 = np.ascontiguousarray(
            _bf16_bits(woT).reshape(NPAIR, 128, DM).transpose(1, 0, 2))
        wqk_fold = _fold(np.ascontiguousarray(np.concatenate([wq, wk], 0).T))
        # [128, KT, 1024] -> f-chunk-major [128, 8, KT, 128]
        wqk_fold = np.ascontiguousarray(
            wqk_fold.reshape(128, KT, 8, 128).transpose(0, 2, 1, 3))
        in_maps.append({
            "xT": xTb[b],
            "wqkT": wqk_fold,
            "wvT": _fold(np.ascontiguousarray(wv.T)),
            "wo16": wo16,
        })
    return in_maps


def kernel(x, w_qkv, w_out, b_out, _trace=False):
    x = np.asarray(x, dtype=np.float32)
    w_qkv = np.asarray(w_qkv, dtype=np.float32)
    w_out = np.asarray(w_out, dtype=np.float32)
    b_out = np.asarray(b_out, dtype=np.float32)

    nc = _get_nc()
    in_maps = make_in_maps(x, w_qkv, w_out)
    res = bass_utils.run_bass_kernel_spmd(
        nc, in_maps, core_ids=list(range(N_CORES)), trace=_trace)
    out = np.empty((4, TOK, DM), dtype=np.float32)
    for b in range(4):
        out[b] = res.results[2 * b]["out_p"] + res.results[2 * b + 1]["out_p"]
    out += b_out
    if _trace:
        kernel.last_results = res
    return out



# revision 65
# speedup vs baseline: 1.0588x; 1.0060x over previous
"""Multi-head attention (b=4, n=2048, dm=1024, h=16) on 8 TRN2 NeuronCores.

Sharding: batch (4) x head-group (2) -> 8 cores, Megatron-style.
Core c handles batch c//2 and heads [8*(c%2), 8*(c%2)+8).

v6 design (HW 367us; v5 380us, v3 440us, v1 616us).  Trace-driven findings:
  * Stage 2 is jointly limited by PE matmul streaming (~225us of
    N-cycle work), exp throughput (33.5M elements across scalar+DVE),
    and the scores->exp->scores PSUM-slot dependency ring.  v3 ran the
    ring with 2 slots and 4:1 exp split: warm clock but exp-bound
    (~300us).  v5 uses 256-query iterations so a score psum bank holds
    [128, 2 heads x 256q] and a [128,1024] 2-bank tile covers TWO
    key-chunks: exp stays a big paired instruction (engine overhead
    ~220ns amortized over 1024 cols) AND the score pool holds 3 slots
    (6 banks) + 1-bank ctx accumulators x2 = 8 banks exactly.  Scores
    only wait on exp three pairs back; ctx matmuls trail scores by two
    pairs (deeper trailing measured worse).  The whole stage runs as
    ONE flat 256-pair software pipeline: each iteration's last two ctx
    groups and its denominator chain wrap into the following iteration,
    so the PE never drains through the exp tail at a boundary (-14us).
  * exp split 5:3 scalar:DVE (bf16 Schraudolph int16 bit trick on DVE),
    interleaved s,s,d,s,d,s,s,d so the scalar never builds a backlog.
    Every softmax row sees a uniform 3/8 trick share (rel err 1.03e-2
    vs the 2e-2 gate; all-fp8/DoubleRow variants measured 2-5e-2 in a
    host-side quantization study -- attention output here is a
    near-uniform average, std ~1/45 of v, so e4m3 noise lands ~1:1 on
    the output; fp8 is unusable everywhere in this problem).
  * Softmax denominator: ones column FIRST in v' (padded to 128 cols:
    [ones, 63 zeros, v0..63]) so the denominator lands on psum
    PARTITION 0 -- the custom-DVE reciprocal_approx_fast and the
    gpsimd partition_broadcast ucode both silently require base
    partition 0 on real HW (verified; partition-64 inputs return
    garbage).  Chain per iteration: DVE row copy + reciprocal, gpsimd
    broadcast to all 128 partitions, then two DVE multiplies (psum rows
    64:128, a legal DVE base-64 window) + SBUF->SBUF partition-shift
    DMAs into ctx_sb.  Nothing here touches the PE or the score pool,
    and the chain is short enough that the multiplies never wait (a
    late rec head-of-line-blocks the DVE exp queue -- measured 2us
    stalls with the earlier DMA-shift chain).
  * HW gotchas baked in: two CONCURRENT row-tiled matmuls (the K=64
    head pair at tile rows 0/64) must write DIFFERENT psum banks or
    the device hangs; one psum accumulation group per 2KB bank
    (zero-region granularity); each dma_start costs ~3us of queue
    time, so stage-1 inputs ship as a few fat DMAs (x quarter in one,
    weights f-chunk-major so matmul #1 only waits for x + 1/8 of wqk).
  * Stage 1 f32r QKV matmuls from bf16-bit inputs; psum pairs
    [128,1024] so each psum->bf16 cast covers two matmul outputs,
    alternating scalar/DVE.  Stage 3 bf16 with one 2-bank psum per
    token block, consecutive matmuls sharing their stationary ctx
    chunk, single [128,1024] cast, one output DMA per block.
Host sums the two partials per batch and adds the bias.
"""

import numpy as np

import concourse.bass as bass
import concourse.tile as tile
from concourse import bacc, library_config, mybir
from concourse import bass_utils

f32 = mybir.dt.float32
f32r = mybir.dt.float32r
f8 = mybir.dt.float8e4
bf16 = mybir.dt.bfloat16
i16 = mybir.dt.int16
u16 = mybir.dt.uint16
Exp = mybir.ActivationFunctionType.Exp
Copy = mybir.ActivationFunctionType.Copy
Mult = mybir.AluOpType.mult
Add = mybir.AluOpType.add

TOK = 2048          # tokens per batch
DM = 1024           # model dim
DL = 512            # local q/k/v feature dim (8 heads x 64)
D = 64              # head dim
NH = 8              # local heads
NPAIR = 4           # head pairs (partition blocks of ctx/qk)
KT = 8              # dm / 128 contraction tiles
SCALE = DM ** (-0.5)
N_CORES = 8

# bf16 Schraudolph: bf16bits(exp(s*SCALE)) ~= trunc(s * A16 + B16).
# The -6.75 zeroes the trick's mean multiplicative bias (so scalar-exp'd
# and trick'd key-chunks weight consistently inside one softmax row);
# it splits the difference between truncating (-6.5) and rounding (-7.0)
# f32->i16 conversion.
A16 = 128.0 * SCALE / float(np.log(2.0))
B16 = 127.0 * 128.0 - 6.75

# exp engine schedule, cycled per PAIRED exp tile ([128,1024] covering
# TWO key-chunks x both heads at 256-query granularity): 5/8 scalar,
# 3/8 DVE, which balances the engines (DVE also carries the softmax
# division) and keeps every softmax row's Schraudolph share a uniform
# 3/8.  (gpsimd cannot access PSUM on TRN2, so it cannot help w/ exp.)
EXP_SCHED = ("s", "s", "d", "s", "d", "s", "s", "d")


def _build(tc, xT, wqkT, wvT, wo16, out_p):
    nc = tc.nc
    # gpsimd ucode library with partition_broadcast (softmax denominator)
    nc.gpsimd.load_library(library_config.attn)

    w3p = tc.alloc_tile_pool(name="w3", bufs=1)
    ctp = tc.alloc_tile_pool(name="ctp", bufs=1)
    qkp = tc.alloc_tile_pool(name="qkp", bufs=1)
    vp_ = tc.alloc_tile_pool(name="vp", bufs=1)

    ctx_sb = ctp.tile([128, NPAIR, TOK], bf16, tag="ctx")    # 16 KB/part
    qk_sb = qkp.tile([128, 2 * NPAIR, TOK], bf16, tag="qk")  # 32 KB/part
    # v' columns: [ones, 63 zero-pad, v0..v63] = 128.  The ones column in
    # position 0 puts the softmax denominator on psum PARTITION 0 (legal
    # base for the custom-DVE reciprocal and the gpsimd broadcast, which
    # silently require it on HW) while the ctx rows land at 64:128 (legal
    # DVE base-64 64-partition window for the division multiplies).
    v_sb = vp_.tile([128, 16, NH, 128], bf16, tag="v")       # 32 KB/part
    # output-projection weights prefetched up front so stage 3 never
    # waits on DMA
    wout_sb = w3p.tile([128, NPAIR, DM], u16, tag="wout")    # 8 KB/part
    nc.sync.dma_start(wout_sb[:], wo16[:])

    # ones column FIRST in v' (softmax denominator accumulator): the
    # denominator then lands on psum PARTITION 0, where the custom-DVE
    # reciprocal and the gpsimd partition_broadcast actually work on HW
    # (both silently require base partition 0) -- no DMA shift needed
    # on the latency-critical reciprocal chain
    nc.vector.memset(v_sb[:, :, :, 0:1], 1.0)
    nc.vector.memset(v_sb[:, :, :, 1:64], 0.0)

    # ---- stage 1: QKV projection in f32r, token-quarters of 512 ----
    # psum tiles are PAIRED [128,1024] (2 banks) so every psum->bf16 cast
    # covers two matmul outputs: the ~220ns per-instruction engine
    # overhead is paid half as often on both DVE and scalar.
    with (
        tc.tile_pool(name="w1", bufs=1) as wp,
        tc.tile_pool(name="x", bufs=2) as xp,
        tc.tile_pool(name="psq", bufs=2, space="PSUM") as psq,
    ):
        # wqkT is staged f-chunk-major [128, 8(f), KT, 128] so the first
        # q/k matmul group only waits for x plus 1/8 of the weights
        wqk_sb = wp.tile([128, 8, KT, 128], u16, tag="wqk")   # 16 KB/part
        wv_sb = wp.tile([128, KT, DL], u16, tag="wv")         # 8 KB/part
        xt0 = xp.tile([128, KT, 512], u16, tag="x")
        # few, fat DMAs: each dma_start costs ~3us of queue time here, so
        # batch everything except the f0 weight chunk (which gates MM #1)
        nc.sync.dma_start(xt0[:], xT[:, :, 0:512])
        nc.sync.dma_start(wqk_sb[:, 0], wqkT[:, 0])
        nc.sync.dma_start(wqk_sb[:, 1:8], wqkT[:, 1:8])
        nc.sync.dma_start(wv_sb[:], wvT[:])
        ncast = 0
        for tq in range(4):
            ts512 = slice(tq * 512, (tq + 1) * 512)
            if tq == 0:
                xt = xt0
            else:
                xt = xp.tile([128, KT, 512], u16, tag="x")
                nc.sync.dma_start(xt[:], xT[:, :, ts512])
            # q/k (transposed layout): out [feat 128, tok 512], f-pairs
            for fp in range(4):
                ps = psq.tile([128, 1024], f32, tag="qk", bufs=3)
                for half in range(2):
                    f = 2 * fp + half
                    hs = slice(half * 512, (half + 1) * 512)
                    for kt in range(KT):
                        nc.tensor.matmul(
                            ps[:, hs],
                            wqk_sb[:, f, kt, :].bitcast(bf16),
                            xt[:, kt, :].bitcast(bf16),
                            start=(kt == 0), stop=(kt == KT - 1))
                with nc.allow_low_precision(reason="bf16"):
                    dst = qk_sb[:, 2 * fp:2 * fp + 2, ts512]
                    src = ps[:].rearrange("p (two t) -> p two t", two=2)
                    if ncast % 2 == 0:
                        nc.vector.tensor_copy(dst, src)
                    else:
                        nc.scalar.activation(dst, src, Copy)
                ncast += 1
            # v (natural layout): out [tok 128, feat 512], tt-pairs
            for tp in range(2):
                ps = psq.tile([128, 1024], f32, tag="v", bufs=1)
                for half in range(2):
                    tt = 2 * tp + half
                    hs = slice(half * 512, (half + 1) * 512)
                    for kt in range(KT):
                        nc.tensor.matmul(
                            ps[:, hs],
                            xt[:, kt, tt * 128:(tt + 1) * 128].bitcast(bf16),
                            wv_sb[:, kt, :].bitcast(bf16),
                            start=(kt == 0), stop=(kt == KT - 1))
                jt0 = tq * 4 + 2 * tp
                with nc.allow_low_precision(reason="bf16 v"):
                    nc.vector.tensor_copy(
                        v_sb[:, jt0:jt0 + 2, :, 64:128],
                        ps[:].rearrange("p (two h d) -> p two h d",
                                        two=2, h=NH))

    # ---- stage 2: attention, per head-pair, query-eighths of 256 ----
    with (
        tc.tile_pool(name="psS", bufs=3, space="PSUM") as psS,
        tc.tile_pool(name="psC", bufs=2, space="PSUM") as psC,
        tc.tile_pool(name="ep", bufs=8) as ep,
        tc.tile_pool(name="dv", bufs=2) as dv,
    ):
        eidx = 0

        def div_tail(ps_c, rec, ihs, p):
            # softmax division for both heads of the pair.  rec (rows
            # 64:128 = broadcast reciprocal denominators) produced at
            # the end of the accumulating iteration with a short chain
            # (DVE copy + DVE recip + gpsimd broadcast, all at their
            # native partitions), so these multiplies never wait and
            # never head-of-line-block the DVE's exp queue.  ctx lives
            # on psum rows 64:128; both heads shift partitions via cheap
            # off-critical-path SBUF->SBUF DMAs.
            with nc.allow_low_precision(reason="bf16 ctx"):
                for hh in (0, 1):
                    tmp = dv.tile([128, 256], bf16, tag="tmp",
                                  name=f"tmp{hh}")
                    nc.vector.tensor_tensor(
                        out=tmp[64:128, :],
                        in0=ps_c[64:128, hh * 256:(hh + 1) * 256],
                        in1=rec[64:128, hh * 256:(hh + 1) * 256], op=Mult)
                    nc.sync.dma_start(
                        ctx_sb[64 * hh:64 * hh + 64, p, ihs], tmp[64:128, :])

        pending_div = None   # (ps_c, rec, ihs, p) awaiting division
        hist = []            # (ps_c, p, ihs, jp, e_t) with pending ctx

        def ctx_mm(ps_c_t, p_t, jp_c, e_kc):
            # ONE psum accumulation group for the whole bank (zero-region
            # granularity is a 2KB bank): start on the very first matmul,
            # stop on the very last.  Full-height lhsT -> the ctx matmuls
            # never run concurrently, so same-bank writes are safe.
            for sub in (0, 1):
                kc = 2 * jp_c + sub
                for hh in (0, 1):
                    nc.tensor.matmul(
                        ps_c_t[:, hh * 256:(hh + 1) * 256],
                        v_sb[:, kc, 2 * p_t + hh, :],
                        e_kc[:, hh * 512 + sub * 256:
                             hh * 512 + (sub + 1) * 256],
                        start=(kc == 0 and hh == 0),
                        stop=(kc == 15 and hh == 1))

        def den_chain(ps_c_t, p_t, ihs_t):
            # denominators sit on psum ROW 0 (ones column is first in
            # v'): copy out, reciprocal, and broadcast all operate at
            # base partition 0 natively -- short low-latency chain
            den0 = dv.tile([1, 512], f32, tag="den0")
            nc.vector.tensor_copy(den0[:], ps_c_t[0:1, :])
            rec0 = dv.tile([1, 512], f32, tag="rec0")
            nc.vector.reciprocal_approx_fast(rec0[:], den0[:])
            rec = dv.tile([128, 512], f32, tag="rec")
            nc.gpsimd.partition_broadcast(rec[:], rec0[:], channels=128)
            return (ps_c_t, rec, ihs_t, p_t)

        # one flat 256-pair software pipeline over (p, i8, jp): the two
        # trailing ctx matmul groups of each (p, i8) iteration wrap into
        # the next iteration instead of draining at its end, so the PE
        # never sits through the exp tail at an iteration boundary
        ps_c = None
        for g in range(256):
            p, i8, jp = g // 64, (g // 8) % 8, g % 8
            ihs = slice(i8 * 256, (i8 + 1) * 256)
            if jp == 0:
                # both heads' ctx accumulators in one 1-bank psum tile
                ps_c = psC.tile([128, 512], f32, tag="C")
            # bank 0 = head a (sub0|sub1 columns), bank 1 = head b: the
            # two heads' row-tiled matmuls run CONCURRENTLY in the PE,
            # so they must write different psum banks (a same-bank
            # concurrent write hangs the device); the same-head/same-
            # bank sub matmuls form one group each
            ps_s = psS.tile([128, 1024], f32, tag="S")
            for sub in (0, 1):
                jt = 2 * jp + sub
                js = slice(jt * 128, (jt + 1) * 128)
                for poff, hh in ((0, 0), (64, 1)):
                    nc.tensor.matmul(
                        ps_s[:, hh * 512 + sub * 256:
                             hh * 512 + (sub + 1) * 256],
                        qk_sb[poff:poff + D, 4 + p, js],
                        qk_sb[poff:poff + D, p, ihs],
                        start=(sub == 0), stop=(sub == 1))
            if len(hist) == 2:
                cps, cp, cihs, cjp, ce = hist.pop(0)
                ctx_mm(cps, cp, cjp, ce)
                if cjp == 7:
                    # that iteration's ctx psum is now complete: launch
                    # its denominator/reciprocal chain immediately
                    pending_div = den_chain(cps, cp, cihs)
            if jp == 5 and pending_div is not None:
                div_tail(*pending_div)
                pending_div = None
            e_t = ep.tile([128, 1024], bf16, tag="E")
            eng = EXP_SCHED[eidx % len(EXP_SCHED)]
            eidx += 1
            with nc.allow_low_precision(reason="exp"):
                if eng == "s":
                    nc.scalar.activation(e_t[:], ps_s[:], Exp,
                                         scale=SCALE)
                else:
                    nc.vector.tensor_scalar(
                        e_t[:].bitcast(i16), ps_s[:],
                        A16, B16, Mult, Add)
            hist.append((ps_c, p, ihs, jp, e_t))
        # final drain: last two pairs' ctx, then the last den chain
        for cps, cp, cihs, cjp, ce in hist:
            ctx_mm(cps, cp, cjp, ce)
        pending_div = den_chain(ps_c, 3, slice(7 * 256, 8 * 256))
        div_tail(*pending_div)

    vp_.release()
    qkp.release()

    # ---- stage 3: output projection (bf16) ----
    # one 2-bank psum per token-block; consecutive matmuls share their
    # stationary ctx chunk, and the psum->f32 cast is a single [128,1024]
    # instruction alternating between scalar and DVE
    with (
        tc.tile_pool(name="psO", bufs=2, space="PSUM") as psO,
        tc.tile_pool(name="ot", bufs=3) as otp,
    ):
        for tt in range(16):
            o_t = otp.tile([128, DM], f32, tag="o")
            ps = psO.tile([128, 1024], f32, tag="O")
            for pb in range(NPAIR):
                for fc in range(2):
                    fs = slice(fc * 512, (fc + 1) * 512)
                    nc.tensor.matmul(
                        ps[:, fs], ctx_sb[:, pb, tt * 128:(tt + 1) * 128],
                        wout_sb[:, pb, fs].bitcast(bf16),
                        start=(pb == 0), stop=(pb == NPAIR - 1))
            if tt % 2 == 0:
                nc.scalar.activation(o_t[:], ps[:], Copy)
            else:
                nc.vector.tensor_copy(o_t[:], ps[:])
            nc.sync.dma_start(out_p[tt * 128:(tt + 1) * 128, :], o_t[:])
    ctp.release()
    w3p.release()


_CACHE = {}


def _get_nc():
    if "nc" not in _CACHE:
        nc = bacc.Bacc("TRN2", target_bir_lowering=False, debug=False)
        xT = nc.dram_tensor("xT", [128, KT, TOK], u16, kind="ExternalInput").ap()
        wqkT = nc.dram_tensor("wqkT", [128, 8, KT, 128], u16,
                              kind="ExternalInput").ap()
        wvT = nc.dram_tensor("wvT", [128, KT, DL], u16,
                             kind="ExternalInput").ap()
        wo16 = nc.dram_tensor("wo16", [128, NPAIR, DM], u16,
                              kind="ExternalInput").ap()
        out_p = nc.dram_tensor("out_p", [TOK, DM], f32, kind="ExternalOutput").ap()
        with tile.TileContext(nc) as tc:
            _build(tc, xT, wqkT, wvT, wo16, out_p)
        nc.compile()
        _CACHE["nc"] = nc
    return _CACHE["nc"]


def _bf16_bits(x):
    b = np.ascontiguousarray(np.asarray(x, np.float32)).view(np.uint32)
    return ((b + 0x7FFF + ((b >> 16) & 1)) >> 16).astype(np.uint16)


def _fold(a):
    """[DM, cols] -> [128, KT, cols] bf16 bits, partition-major dm chunks."""
    b = _bf16_bits(a)
    return np.ascontiguousarray(
        b.reshape(KT, 128, a.shape[1]).transpose(1, 0, 2))


def make_in_maps(x, w_qkv, w_out):
    in_maps = []
    xTb = {b: _fold(np.ascontiguousarray(x[b].T)) for b in range(4)}
    for c in range(N_CORES):
        b, g = c // 2, c % 2
        gs = slice(g * DL, (g + 1) * DL)
        wq = w_qkv[0 * DM + g * DL:0 * DM + (g + 1) * DL]
        wk = w_qkv[1 * DM + g * DL:1 * DM + (g + 1) * DL]
        wv = w_qkv[2 * DM + g * DL:2 * DM + (g + 1) * DL]
        woT = np.ascontiguousarray(w_out[:, gs].T)        # [DL, DM]
        wo16 = np.ascontiguousarray(
            _bf16_bits(woT).reshape(NPAIR, 128, DM).transpose(1, 0, 2))
        wqk_fold = _fold(np.ascontiguousarray(np.concatenate([wq, wk], 0).T))
        # [128, KT, 1024] -> f-chunk-major [128, 8, KT, 128]
        wqk_fold = np.ascontiguousarray(
            wqk_fold.reshape(128, KT, 8, 128).transpose(0, 2, 1, 3))
        in_maps.append({
            "xT": xTb[b],
            "wqkT": wqk_fold,
            "wvT": _fold(np.ascontiguousarray(wv.T)),
            "wo16": wo16,
        })
    return in_maps


def kernel(x, w_qkv, w_out, b_out, _trace=False):
    x = np.asarray(x, dtype=np.float32)
    w_qkv = np.asarray(w_qkv, dtype=np.float32)
    w_out = np.asarray(w_out, dtype=np.float32)
    b_out = np.asarray(b_out, dtype=np.float32)

    nc = _get_nc()
    in_maps = make_in_maps(x, w_qkv, w_out)
    res = bass_utils.run_bass_kernel_spmd(
        nc, in_maps, core_ids=list(range(N_CORES)), trace=_trace)
    out = np.empty((4, TOK, DM), dtype=np.float32)
    for b in range(4):
        out[b] = res.results[2 * b]["out_p"] + res.results[2 * b + 1]["out_p"]
    out += b_out
    if _trace:
        kernel.last_results = res
    return out

